# revision 2
# baseline (speedup 1.0000x reference)
"""Swin-style windowed attention (b=16, n=1024, 8 heads x 32, relative
position bias) for 8 Trainium2 NeuronCores, data-parallel over batch.

Software-pipelined redesign:
  - all-bf16 datapath (x, weights, qk tiles): halves input DMA, enables FWL.
  - flipped PV: the attn-weight tile is the FWL *stationary*, v_aug (with a
    ones column per head) the moving operand, so PV output lands token-major
    [tok, 33] with the softmax denominator in psum column 32.
  - normalize: one DVE reciprocal (16 dens) + one broadcast tensor_mul per
    (hg, b, qh) via stride-0 APs -- no ACT ln/exp, no Exp<->Ln table thrash.
  - out-projection: 16-bit XBAR DMA-transpose of on_sb chunks, two FWL
    matmuls, DVE psum->bf16 copy, DMA out; b_out added on host.
  - cross-rep software pipeline: phase-1 of rep r+1 and phase-3 output
    chains are emitted as thunks with "due units" interleaved into the
    phase-2 stream, so ACT (the exp bottleneck) never starves and no DMA
    with a long semaphore wait ever heads a dispatch queue.

ACT is the floor: 128 exp instructions x [128,1024] ~= 121 us/rep.
"""

import dataclasses
from collections import deque

import numpy as np


def _ensure_path():
    try:
        import concourse.bass  # noqa: F401
    except ImportError:
        import sys

        for p in ("/opt/trn_rl_repo", "/root/.axon_site/_ro/trn_rl_repo"):
            if p not in sys.path:
                sys.path.insert(0, p)


_ensure_path()

import concourse.bass as bass  # noqa: E402
import concourse.tile as tile  # noqa: E402
from concourse import mybir  # noqa: E402
from concourse.bass_utils import run_bass_kernel_spmd  # noqa: E402

HEADS = 8
DH = 32
INP = 512
OUP = 512
N = 1024
B = 16
NCORES = 8
BPC = B // NCORES
T = BPC * N
TABLE = 3969

F32 = mybir.dt.float32
BF = mybir.dt.bfloat16
Exp = mybir.ActivationFunctionType.Exp

_COMPUTE_CAP = 1

UNITS = 128  # phase-2 (hg, b, qh, kt, hp) units per rep


def _split_waits(nc, cap=1):
    """Split instructions with too many semaphore waits into same-engine
    NoOp chains (walrus on this build rejects >1 wait per instruction)."""
    n = 0
    for _, bb_wrap in nc.bb_map.items():
        bb = bb_wrap.bb if hasattr(bb_wrap, "bb") else bb_wrap
        new_list = []
        changed = False
        for inst in bb.instructions:
            si = inst.sync_info
            cap = _COMPUTE_CAP
            if si is not None and si.on_wait and len(si.on_wait) > cap:
                waits = list(si.on_wait)
                rest, head = waits[:-cap], waits[-cap:]
                for i in range(0, len(rest), cap):
                    nop = mybir.InstNoOp(name=f"{inst.name}_wsplit{i}")
                    nop.engine = inst.engine
                    nop.sync_info = mybir.SyncInfo(
                        on_wait=rest[i : i + cap], on_update=[]
                    )
                    nc.register_instruction(nop, overwrite=True)
                    new_list.append(nop)
                    n += 1
                inst.sync_info = mybir.SyncInfo(
                    on_wait=head, on_update=list(si.on_update)
                )
                changed = True
            new_list.append(inst)
        if changed:
            bb.instructions = new_list
    return n


class _Pipe:
    """Due-unit thunk scheduler: thunks are emitted into the instruction
    stream once the global phase-2 unit counter reaches their due unit."""

    def __init__(self):
        self.pending = []  # sorted list of (due, seq, thunk)
        self.seq = 0
        self.g = 0

    def push(self, due, thunk):
        self.pending.append((due, self.seq, thunk))
        self.seq += 1
        self.pending.sort(key=lambda x: (x[0], x[1]))

    def tick(self, g):
        self.g = g
        while self.pending and self.pending[0][0] <= g:
            _, _, th = self.pending.pop(0)
            th()

    def flush(self):
        for _, _, th in self.pending:
            th()
        self.pending = []


def _build(nc, tc, es, aps, reps, dbg=None):
    from contextlib import ExitStack  # noqa: F401

    xT_d, wqk_d, wv_d, eb_d, wo_d, y_d = aps

    # ---- persistent pools --------------------------------------------------
    p_eb = es.enter_context(tc.tile_pool(name="eb", bufs=3))
    p_qk = es.enter_context(tc.tile_pool(name="qk", bufs=8))
    p_x = es.enter_context(tc.tile_pool(name="xT", bufs=4))
    p_w = es.enter_context(tc.tile_pool(name="w", bufs=8))
    p_wo = es.enter_context(tc.tile_pool(name="wo", bufs=4))
    p_v = es.enter_context(tc.tile_pool(name="v", bufs=2))
    p_on = es.enter_context(tc.tile_pool(name="on", bufs=1))
    p_p = es.enter_context(tc.tile_pool(name="pexp", bufs=4))
    p_ph = es.enter_context(tc.tile_pool(name="phat", bufs=4))
    p_rec = es.enter_context(tc.tile_pool(name="rec", bufs=2))
    p_ot = es.enter_context(tc.tile_pool(name="ot", bufs=8))
    p_y = es.enter_context(tc.tile_pool(name="ysb", bufs=4))
    p_ps1 = es.enter_context(tc.tile_pool(name="ps1", bufs=2, space="PSUM"))
    p_pd = es.enter_context(tc.tile_pool(name="pd", bufs=2, space="PSUM"))
    p_pv = es.enter_context(tc.tile_pool(name="pv", bufs=1, space="PSUM"))

    pipe = _Pipe()
    st = {}  # per-rep tile refs: st[r] = dict(qk=..., v=..., eb=..., wo=...)

    # ---- emission helpers --------------------------------------------------
    def emit_in_dma(r):
        s = st.setdefault(r, {})
        xT = []
        for dm in range(4):
            t = p_x.tile([128, T], BF, tag="xT", name=f"xT{dm}_r{r}")
            nc.sync.dma_start(t[:], xT_d[dm * 128 : (dm + 1) * 128, :])
            xT.append(t)
        s["xT"] = xT
        wqk, wv = [], []
        for dm in range(4):
            t = p_w.tile([128, 512], BF, tag="wqk", name=f"wqk{dm}_r{r}")
            nc.sync.dma_start(t[:], wqk_d[dm * 128 : (dm + 1) * 128, :])
            wqk.append(t)
            t = p_w.tile([128, 256], BF, tag="wv", name=f"wv{dm}_r{r}")
            nc.sync.dma_start(t[:], wv_d[dm * 128 : (dm + 1) * 128, :])
            wv.append(t)
        s["wqk"], s["wv"] = wqk, wv
        wo = []
        for g in range(2):
            t = p_wo.tile([128, 512], BF, tag="wo", name=f"wo{g}_r{r}")
            nc.sync.dma_start(t[:], wo_d[g])
            wo.append(t)
        s["wo"] = wo

    def emit_eb_dma(r, hg, h2):
        s = st.setdefault(r, {})
        t = p_eb.tile([128, 16384], BF, tag="eb", name=f"eb{hg}_{h2}_r{r}")
        nc.gpsimd.dma_start(t[:], eb_d[hg, h2])
        s[("eb", hg, h2)] = t

    def emit_qk_alloc(r, ft):
        s = st.setdefault(r, {})
        s.setdefault("qk", {})[ft] = p_qk.tile(
            [128, T], BF, tag="qk", name=f"qk{ft}_r{r}"
        )

    def emit_qkv_group(r, ft, tb):
        s = st[r]
        ps = p_ps1.tile([128, 512], F32, tag="ps1", name="psq")
        for dm in range(4):
            nc.tensor.matmul(
                ps[:],
                s["wqk"][dm][:, ft * 128 : (ft + 1) * 128],
                s["xT"][dm][:, tb * 512 : (tb + 1) * 512],
                start=(dm == 0),
                stop=(dm == 3),
            )
        nc.vector.tensor_copy(
            s["qk"][ft][:, tb * 512 : (tb + 1) * 512], ps[:]
        )

    def emit_v_alloc(r):
        s = st.setdefault(r, {})
        v_aug = p_v.tile([128, 16 * 264], BF, tag="v", name=f"v_aug_r{r}")
        s["v"] = v_aug
        va = v_aug[:].rearrange("p (t h c) -> p t h c", t=16, h=8)
        nc.gpsimd.memset(va[:, :, :, 32:33], 1.0)

    def emit_v_group(r, tt):
        s = st[r]
        ps = p_ps1.tile([128, 256], F32, tag="ps1", name="psv")
        for dm in range(4):
            nc.tensor.matmul(
                ps[:],
                s["xT"][dm][:, tt * 128 : (tt + 1) * 128],
                s["wv"][dm][:],
                start=(dm == 0),
                stop=(dm == 3),
            )
        va = s["v"][:].rearrange("p (t h c) -> p t h c", t=16, h=8)
        nc.vector.tensor_copy(va[:, tt, :, 0:32], ps[:].rearrange("p (h d) -> p h d", h=8))

    def emit_on_alloc(r):
        s = st.setdefault(r, {})
        s["on"] = p_on.tile([128, 16 * 256], BF, tag="on", name=f"on_r{r}")

    def emit_p3_transposes(r, b):
        s = st[r]
        ots = s.setdefault("ots", {})
        for tloc in range(8):
            t = b * 8 + tloc
            for g in range(2):
                ot = p_ot.tile([128, 128], BF, tag="ot", name=f"ot{t}_{g}_r{r}")
                nc.sync.dma_start(
                    ot[:],
                    s["on"][:, t * 256 + g * 128 : t * 256 + (g + 1) * 128],
                    transpose=True,
                )
                ots[(t, g)] = ot

    def emit_p3_chain(r, t):
        s = st[r]
        psy = p_ps1.tile([128, 512], F32, tag="ps1", name=f"psy{t}_r{r}")
        for g in range(2):
            nc.tensor.matmul(
                psy[:],
                s["ots"][(t, g)][:],
                s["wo"][g][:],
                start=(g == 0),
                stop=(g == 1),
            )
        ysb = p_y.tile([128, 512], BF, tag="ysb")
        nc.vector.tensor_copy(ysb[:], psy[:])
        nc.sync.dma_start(y_d[t], ysb[:])

    def schedule_rep_prelude(r, base):
        """Schedule rep r's phase-1 + eb thunks into the stream.
        base = global unit index where scheduling may start."""
        pipe.push(base + 10, lambda: emit_in_dma(r))
        pipe.push(base + 20, lambda: emit_v_alloc(r))
        pipe.push(base + 20, lambda: emit_on_alloc(r))
        pipe.push(base + 21, lambda: emit_qk_alloc(r, 0))
        pipe.push(base + 21, lambda: emit_qk_alloc(r, 2))
        due = base + 22
        for tb in range(4):
            pipe.push(due, lambda ft=0, tb=tb: emit_qkv_group(r, ft, tb))
            due += 3
            pipe.push(due, lambda ft=2, tb=tb: emit_qkv_group(r, ft, tb))
            due += 3
        for tt in range(16):
            pipe.push(due, lambda tt=tt: emit_v_group(r, tt))
            due += 3
        # late halves (q/k heads 4-7): slots free after donor rep's hg0
        pipe.push(base + 66, lambda: emit_qk_alloc(r, 1))
        pipe.push(base + 67, lambda: emit_qk_alloc(r, 3))
        due = base + 68
        for tb in range(4):
            pipe.push(due, lambda ft=1, tb=tb: emit_qkv_group(r, ft, tb))
            due += 2
            pipe.push(due, lambda ft=3, tb=tb: emit_qkv_group(r, ft, tb))
            due += 2
        return due

    # ---- rep 0 prologue (direct emission, no pipeline) ---------------------
    emit_in_dma(0)
    emit_eb_dma(0, 0, 0)
    emit_eb_dma(0, 0, 1)
    emit_eb_dma(0, 1, 0)
    emit_v_alloc(0)
    emit_on_alloc(0)
    for ft in range(4):
        emit_qk_alloc(0, ft)
    for ft in range(4):
        for tb in range(4):
            emit_qkv_group(0, ft, tb)
    for tt in range(16):
        emit_v_group(0, tt)

    # ---- main rep loop -----------------------------------------------------
    for r in range(reps):
        R0 = r * UNITS
        if r == 0:
            # rep 0's e11 slot frees once hg0's readers drain
            pipe.push(R0 + 66, lambda r=r: emit_eb_dma(r, 1, 1))
        if r + 1 < reps:
            schedule_rep_prelude(r + 1, R0)
            # next rep's eb tiles: e00' after our e01 frees (hg0 done),
            # e01' after our e10 frees, e10' right after rep boundary,
            # e11' once next rep's hg0 drains.
            pipe.push(R0 + 68, lambda r=r: emit_eb_dma(r + 1, 0, 0))
            pipe.push(R0 + 124, lambda r=r: emit_eb_dma(r + 1, 0, 1))
            pipe.push(R0 + UNITS + 2, lambda r=r: emit_eb_dma(r + 1, 1, 0))
            pipe.push(R0 + UNITS + 66, lambda r=r: emit_eb_dma(r + 1, 1, 1))

        s = st[r]
        g = R0
        for hg in range(2):
            for b in range(BPC):
                for qh in range(2):
                    q0 = b * N + qh * 512
                    pv = p_pv.tile([128, 1024], F32, tag="pv", name=f"pv{hg}{b}{qh}_r{r}")
                    for kt in range(8):
                        half, ktl = kt // 4, kt % 4
                        ebtile = s[("eb", hg, half)]
                        for hp in range(2):
                            pd = p_pd.tile([128, 1024], F32, tag="pd", name="pd")
                            for jj in range(2):
                                j = hp * 2 + jj
                                pb = j * 32
                                nc.tensor.matmul(
                                    pd[:, jj * 512 : (jj + 1) * 512],
                                    s["qk"][2 + hg][pb : pb + 32, b * N + kt * 128 : b * N + kt * 128 + 128],
                                    s["qk"][hg][pb : pb + 32, q0 : q0 + 512],
                                    start=True,
                                    stop=True,
                                    tile_position=(pb, 0),
                                )
                            P = p_p.tile([128, 1024], BF, tag="P")
                            nc.scalar.activation(P[:], pd[:], Exp)
                            Ph = p_ph.tile([128, 1024], BF, tag="Ph")
                            ebsl = ebtile[:].rearrange("p (h q) -> p h q", h=16)[
                                :,
                                ktl * 4 + hp * 2 : ktl * 4 + hp * 2 + 2,
                                qh * 512 : qh * 512 + 512,
                            ]
                            psl = P[:].rearrange("p (h q) -> p h q", h=2)
                            phsl = Ph[:].rearrange("p (h q) -> p h q", h=2)
                            nc.vector.tensor_mul(phsl, psl, ebsl)
                            if (
                                dbg is not None
                                and r == reps - 1
                                and (hg, b, qh, kt, hp) == (0, 0, 0, 0, 0)
                            ):
                                pdsb = p_p.tile([128, 1024], BF, tag="P", name="pdsb")
                                nc.vector.tensor_copy(pdsb[:], pd[:])
                                nc.sync.dma_start(dbg["pd"], pdsb[:])
                                nc.sync.dma_start(dbg["P"], P[:])
                                nc.sync.dma_start(dbg["Ph"], Ph[:])
                            base = (b * 8 + kt) * 264
                            for jj in range(2):
                                j = hp * 2 + jj
                                h = hg * 4 + j
                                for c in range(4):
                                    # pv banks: bank hp holds heads hp*2..hp*2+1.
                                    # start=True only on the FIRST matmul to touch
                                    # a bank: start marks the whole 2KB zero region
                                    # pending-zero, so later footprints' first
                                    # writes overwrite (= init) and then accumulate.
                                    # Verified bit-exact on HW.
                                    nc.tensor.matmul(
                                        pv[:, j * 256 + c * 64 : j * 256 + c * 64 + 33],
                                        Ph[:, jj * 512 + c * 128 : jj * 512 + c * 128 + 128],
                                        s["v"][:, base + h * 33 : base + h * 33 + 33],
                                        start=(kt == 0 and jj == 0 and c == 0),
                                        stop=(kt == 7),
                                        skip_group_check=True,
                                    )
                            g += 1
                            pipe.tick(g)
                    # ---- merged normalize for this (hg, b, qh) -------------
                    if (
                        dbg is not None
                        and r == reps - 1
                        and (hg, b, qh) == (0, 0, 0)
                    ):
                        pvsb = p_p.tile([128, 1024], BF, tag="P", name="pvsb")
                        pv_in = dataclasses.replace(
                            pv[:, 0:33], ap=[list(pv[:].ap[0]), [64, 16], [1, 33]]
                        )
                        pv_out = dataclasses.replace(
                            pvsb[:, 0:33], ap=[list(pvsb[:].ap[0]), [33, 16], [1, 33]]
                        )
                        nc.vector.tensor_copy(pv_out, pv_in)
                        nc.sync.dma_start(dbg["pv"][:, 0:528], pvsb[:, 0:528])
                    rec = p_rec.tile([128, 16], F32, tag="rec")
                    pvap = pv[:]
                    den = dataclasses.replace(
                        pv[:, 32:33], ap=[list(pvap.ap[0]), [64, 4], [256, 4]]
                    )
                    nc.vector.reciprocal(
                        dataclasses.replace(
                            rec[:, 0:16], ap=[list(rec[:].ap[0]), [4, 4], [1, 4]]
                        ),
                        den,
                    )
                    t0c = (b * 8 + qh * 4) * 256 + hg * 128
                    in1 = dataclasses.replace(
                        pv[:, 0:32], ap=[list(pvap.ap[0]), [64, 4], [256, 4], [1, 32]]
                    )
                    in2 = dataclasses.replace(
                        rec[:, 0:16], ap=[list(rec[:].ap[0]), [4, 4], [1, 4], [0, 32]]
                    )
                    out = dataclasses.replace(
                        s["on"][:, t0c : t0c + 32],
                        ap=[list(s["on"][:].ap[0]), [256, 4], [32, 4], [1, 32]],
                    )
                    nc.vector.tensor_mul(out, in1, in2)
                # ---- phase 3 for batch b (after hg==1) ---------------------
                if hg == 1:
                    emit_p3_transposes(r, b)
                    for tloc in range(8):
                        t = b * 8 + tloc
                        if b == 0:
                            pipe.push(R0 + 98 + 2 * tloc, lambda r=r, t=t: emit_p3_chain(r, t))
                        else:
                            pipe.push(R0 + UNITS + 4 + 2 * tloc, lambda r=r, t=t: emit_p3_chain(r, t))
        pipe.tick(R0 + UNITS)
    pipe.flush()
    if dbg is not None:
        s = st[reps - 1]
        nc.sync.dma_start(dbg["on"], s["on"][:])
        nc.sync.dma_start(dbg["v"], s["v"][:])
        for ft in range(4):
            nc.sync.dma_start(dbg["qk"][ft], s["qk"][ft][:])


def build_program(reps=1, debug_outs=False):
    nc = bass.Bass("TRN2", target_bir_lowering=False, debug=False, num_devices=NCORES)
    xT_d = nc.dram_tensor("xT", [INP, T], BF, kind="ExternalInput").ap()
    wqk_d = nc.dram_tensor("wqk", [INP, 512], BF, kind="ExternalInput").ap()
    wv_d = nc.dram_tensor("wv", [INP, 256], BF, kind="ExternalInput").ap()
    eb_d = nc.dram_tensor("eb", [2, 2, 128, 16384], BF, kind="ExternalInput").ap()
    wo_d = nc.dram_tensor("wo", [2, 128, 512], BF, kind="ExternalInput").ap()
    y_d = nc.dram_tensor("y", [16, 128, OUP], BF, kind="ExternalOutput").ap()
    aps = (xT_d, wqk_d, wv_d, eb_d, wo_d, y_d)
    dbg = None
    if debug_outs:
        dbg = {
            "on": nc.dram_tensor("dbg_on", [128, 16 * 256], BF, kind="ExternalOutput").ap(),
            "v": nc.dram_tensor("dbg_v", [128, 16 * 264], BF, kind="ExternalOutput").ap(),
            "qk": [
                nc.dram_tensor(f"dbg_qk{i}", [128, T], BF, kind="ExternalOutput").ap()
                for i in range(4)
            ],
            "pd": nc.dram_tensor("dbg_pd", [128, 1024], BF, kind="ExternalOutput").ap(),
            "P": nc.dram_tensor("dbg_P", [128, 1024], BF, kind="ExternalOutput").ap(),
            "Ph": nc.dram_tensor("dbg_Ph", [128, 1024], BF, kind="ExternalOutput").ap(),
            "pv": nc.dram_tensor("dbg_pv", [128, 1024], BF, kind="ExternalOutput").ap(),
        }

    from contextlib import ExitStack

    with tile.TileContext(nc) as tc:
        with ExitStack() as es:
            _build(nc, tc, es, aps, reps, dbg=dbg)

    _split_waits(nc, cap=1)
    return nc


def _relative_index():
    ii, jj = np.meshgrid(np.arange(32), np.arange(32), indexing="ij")
    coords = np.stack([ii.reshape(-1), jj.reshape(-1)])
    rel = coords[:, :, None] - coords[:, None, :]
    return ((rel[0] + 31) * 63 + (rel[1] + 31)).reshape(-1)


def prepare_inputs(x, w_qkv, bias_table, w_out, b_out):
    """Host-side prep: returns per-core in_maps."""
    import ml_dtypes

    bf16 = ml_dtypes.bfloat16
    scale = DH ** -0.5

    wqk = np.ascontiguousarray(w_qkv[:, :512]).astype(np.float32).copy()
    wqk[:, :256] *= scale
    wqk = wqk.astype(bf16)
    wv = np.ascontiguousarray(w_qkv[:, 512:]).astype(bf16)

    idx = np.clip(_relative_index(), 0, TABLE - 1)
    bias = bias_table[idx].reshape(N, N, HEADS).astype(np.float32)  # [q, k, h]
    ebT = np.exp(bias).transpose(1, 0, 2)  # [k, q, h]
    eb = (
        ebT.reshape(2, 4, 128, N, 2, 4)  # [half, ktl, p, qt, hg, j]
        .transpose(4, 0, 2, 1, 5, 3)  # [hg, half, p, ktl, j, qt]
        .reshape(2, 2, 128, 16384)
        .astype(bf16)
    )

    wo = np.ascontiguousarray(w_out.astype(bf16)).reshape(2, 128, OUP)

    in_maps = []
    for c in range(NCORES):
        xc = x[c * BPC : (c + 1) * BPC].reshape(T, INP)
        xT = np.ascontiguousarray(xc.T).astype(bf16)
        in_maps.append({"xT": xT, "wqk": wqk, "wv": wv, "eb": eb, "wo": wo})
    return in_maps


_NC_CACHE = {}


def kernel(x, w_qkv, bias_table, w_out, b_out):
    in_maps = prepare_inputs(x, w_qkv, bias_table, w_out, b_out)
    if 1 not in _NC_CACHE:
        _NC_CACHE[1] = build_program(reps=1)
    nc = _NC_CACHE[1]
    res = run_bass_kernel_spmd(nc, in_maps, list(range(NCORES)), trace=False)
    y = np.concatenate(
        [
            res.results[c]["y"].astype(np.float32).reshape(T, OUP)
            for c in range(NCORES)
        ],
        axis=0,
    ).reshape(B, N, OUP)
    return (y + np.asarray(b_out, np.float32)[None, None, :]).astype(np.float32)


# revision 3
# speedup vs baseline: 1.0463x; 1.0463x over previous
"""Swin-style windowed attention (b=16, n=1024, 8 heads x 32, relative
position bias) for 8 Trainium2 NeuronCores, data-parallel over batch.

Software-pipelined redesign:
  - all-bf16 datapath (x, weights, qk tiles): halves input DMA, enables FWL.
  - flipped PV: the attn-weight tile is the FWL *stationary*, v_aug (with a
    ones column per head) the moving operand, so PV output lands token-major
    [tok, 33] with the softmax denominator in psum column 32.
  - normalize: one DVE reciprocal (16 dens) + one broadcast tensor_mul per
    (hg, b, qh) via stride-0 APs -- no ACT ln/exp, no Exp<->Ln table thrash.
  - out-projection: 16-bit XBAR DMA-transpose of on_sb chunks, two FWL
    matmuls, DVE psum->bf16 copy, DMA out; b_out added on host.
  - cross-rep software pipeline: phase-1 of rep r+1 and phase-3 output
    chains are emitted as thunks with "due units" interleaved into the
    phase-2 stream, so ACT (the exp bottleneck) never starves and no DMA
    with a long semaphore wait ever heads a dispatch queue.

ACT is the floor: 128 exp instructions x [128,1024] ~= 121 us/rep busy
(1 elem/cycle/lane @ 1.2 GHz, no dtype speedup). Measured: 180 us/rep on
HW (burst + reps-in-NEFF differencing) vs 292 us/rep for the previous
kernel on the same methodology; cost-model sim says 151 vs 289.

PSUM note (verified bit-exact on HW): matmul start_tensor_calc marks the
whole 2 KB zero region (bank row) pending-zero, so interleaved
accumulation streams in one bank need start=True ONLY on the first
matmul to touch the bank; later footprints' first writes hit
pending-zero bytes and overwrite (init), then accumulate.
"""

import dataclasses
from collections import deque

import numpy as np


def _ensure_path():
    try:
        import concourse.bass  # noqa: F401
    except ImportError:
        import sys

        for p in ("/opt/trn_rl_repo", "/root/.axon_site/_ro/trn_rl_repo"):
            if p not in sys.path:
                sys.path.insert(0, p)


_ensure_path()

import concourse.bass as bass  # noqa: E402
import concourse.tile as tile  # noqa: E402
from concourse import mybir  # noqa: E402
from concourse.bass_utils import run_bass_kernel_spmd  # noqa: E402

HEADS = 8
DH = 32
INP = 512
OUP = 512
N = 1024
B = 16
NCORES = 8
BPC = B // NCORES
T = BPC * N
TABLE = 3969

F32 = mybir.dt.float32
BF = mybir.dt.bfloat16
Exp = mybir.ActivationFunctionType.Exp

_COMPUTE_CAP = 1

UNITS = 128  # phase-2 (hg, b, qh, kt, hp) units per rep


def _split_waits(nc, cap=1):
    """Split instructions with too many semaphore waits into same-engine
    NoOp chains (walrus on this build rejects >1 wait per instruction)."""
    n = 0
    for _, bb_wrap in nc.bb_map.items():
        bb = bb_wrap.bb if hasattr(bb_wrap, "bb") else bb_wrap
        new_list = []
        changed = False
        for inst in bb.instructions:
            si = inst.sync_info
            cap = _COMPUTE_CAP
            if si is not None and si.on_wait and len(si.on_wait) > cap:
                waits = list(si.on_wait)
                rest, head = waits[:-cap], waits[-cap:]
                for i in range(0, len(rest), cap):
                    nop = mybir.InstNoOp(name=f"{inst.name}_wsplit{i}")
                    nop.engine = inst.engine
                    nop.sync_info = mybir.SyncInfo(
                        on_wait=rest[i : i + cap], on_update=[]
                    )
                    nc.register_instruction(nop, overwrite=True)
                    new_list.append(nop)
                    n += 1
                inst.sync_info = mybir.SyncInfo(
                    on_wait=head, on_update=list(si.on_update)
                )
                changed = True
            new_list.append(inst)
        if changed:
            bb.instructions = new_list
    return n


class _Pipe:
    """Due-unit thunk scheduler: thunks are emitted into the instruction
    stream once the global phase-2 unit counter reaches their due unit."""

    def __init__(self):
        self.pending = []  # sorted list of (due, seq, thunk)
        self.seq = 0
        self.g = 0

    def push(self, due, thunk):
        self.pending.append((due, self.seq, thunk))
        self.seq += 1
        self.pending.sort(key=lambda x: (x[0], x[1]))

    def tick(self, g):
        self.g = g
        while self.pending and self.pending[0][0] <= g:
            _, _, th = self.pending.pop(0)
            th()

    def flush(self):
        for _, _, th in self.pending:
            th()
        self.pending = []


def _build(nc, tc, es, aps, reps, dbg=None):
    from contextlib import ExitStack  # noqa: F401

    xT_d, wqk_d, wv_d, eb_d, wo_d, y_d = aps

    # ---- persistent pools --------------------------------------------------
    p_eb = es.enter_context(tc.tile_pool(name="eb", bufs=3))
    p_qk = es.enter_context(tc.tile_pool(name="qk", bufs=8))
    p_x = es.enter_context(tc.tile_pool(name="xT", bufs=4))
    p_w = es.enter_context(tc.tile_pool(name="w", bufs=8))
    p_wo = es.enter_context(tc.tile_pool(name="wo", bufs=4))
    p_v = es.enter_context(tc.tile_pool(name="v", bufs=2))
    p_on = es.enter_context(tc.tile_pool(name="on", bufs=1))
    p_p = es.enter_context(tc.tile_pool(name="pexp", bufs=4))
    p_ph = es.enter_context(tc.tile_pool(name="phat", bufs=4))
    p_rec = es.enter_context(tc.tile_pool(name="rec", bufs=2))
    p_ot = es.enter_context(tc.tile_pool(name="ot", bufs=8))
    p_y = es.enter_context(tc.tile_pool(name="ysb", bufs=4))
    p_ps1 = es.enter_context(tc.tile_pool(name="ps1", bufs=2, space="PSUM"))
    p_pd = es.enter_context(tc.tile_pool(name="pd", bufs=2, space="PSUM"))
    p_pv = es.enter_context(tc.tile_pool(name="pv", bufs=1, space="PSUM"))

    pipe = _Pipe()
    st = {}  # per-rep tile refs: st[r] = dict(qk=..., v=..., eb=..., wo=...)

    # ---- emission helpers --------------------------------------------------
    def emit_in_dma(r):
        s = st.setdefault(r, {})
        xT = []
        for dm in range(4):
            t = p_x.tile([128, T], BF, tag="xT", name=f"xT{dm}_r{r}")
            nc.sync.dma_start(t[:], xT_d[dm * 128 : (dm + 1) * 128, :])
            xT.append(t)
        s["xT"] = xT
        wqk, wv = [], []
        for dm in range(4):
            t = p_w.tile([128, 512], BF, tag="wqk", name=f"wqk{dm}_r{r}")
            nc.sync.dma_start(t[:], wqk_d[dm * 128 : (dm + 1) * 128, :])
            wqk.append(t)
            t = p_w.tile([128, 256], BF, tag="wv", name=f"wv{dm}_r{r}")
            nc.sync.dma_start(t[:], wv_d[dm * 128 : (dm + 1) * 128, :])
            wv.append(t)
        s["wqk"], s["wv"] = wqk, wv
        wo = []
        for g in range(2):
            t = p_wo.tile([128, 512], BF, tag="wo", name=f"wo{g}_r{r}")
            nc.sync.dma_start(t[:], wo_d[g])
            wo.append(t)
        s["wo"] = wo

    def emit_eb_dma(r, hg, h2):
        s = st.setdefault(r, {})
        t = p_eb.tile([128, 16384], BF, tag="eb", name=f"eb{hg}_{h2}_r{r}")
        nc.gpsimd.dma_start(t[:], eb_d[hg, h2])
        s[("eb", hg, h2)] = t

    def emit_qk_alloc(r, ft):
        s = st.setdefault(r, {})
        s.setdefault("qk", {})[ft] = p_qk.tile(
            [128, T], BF, tag="qk", name=f"qk{ft}_r{r}"
        )

    def emit_qkv_group(r, ft, tb):
        s = st[r]
        ps = p_ps1.tile([128, 512], F32, tag="ps1", name="psq")
        for dm in range(4):
            nc.tensor.matmul(
                ps[:],
                s["wqk"][dm][:, ft * 128 : (ft + 1) * 128],
                s["xT"][dm][:, tb * 512 : (tb + 1) * 512],
                start=(dm == 0),
                stop=(dm == 3),
            )
        nc.vector.tensor_copy(
            s["qk"][ft][:, tb * 512 : (tb + 1) * 512], ps[:]
        )

    def emit_v_alloc(r):
        s = st.setdefault(r, {})
        v_aug = p_v.tile([128, 16 * 264], BF, tag="v", name=f"v_aug_r{r}")
        s["v"] = v_aug
        va = v_aug[:].rearrange("p (t h c) -> p t h c", t=16, h=8)
        nc.gpsimd.memset(va[:, :, :, 32:33], 1.0)

    def emit_v_group(r, tt):
        s = st[r]
        ps = p_ps1.tile([128, 256], F32, tag="ps1", name="psv")
        for dm in range(4):
            nc.tensor.matmul(
                ps[:],
                s["xT"][dm][:, tt * 128 : (tt + 1) * 128],
                s["wv"][dm][:],
                start=(dm == 0),
                stop=(dm == 3),
            )
        va = s["v"][:].rearrange("p (t h c) -> p t h c", t=16, h=8)
        nc.vector.tensor_copy(va[:, tt, :, 0:32], ps[:].rearrange("p (h d) -> p h d", h=8))

    def emit_on_alloc(r):
        s = st.setdefault(r, {})
        s["on"] = p_on.tile([128, 16 * 256], BF, tag="on", name=f"on_r{r}")

    def emit_p3_transposes(r, b):
        s = st[r]
        ots = s.setdefault("ots", {})
        for tloc in range(8):
            t = b * 8 + tloc
            for g in range(2):
                ot = p_ot.tile([128, 128], BF, tag="ot", name=f"ot{t}_{g}_r{r}")
                nc.sync.dma_start(
                    ot[:],
                    s["on"][:, t * 256 + g * 128 : t * 256 + (g + 1) * 128],
                    transpose=True,
                )
                ots[(t, g)] = ot

    def emit_p3_chain(r, t):
        s = st[r]
        psy = p_ps1.tile([128, 512], F32, tag="ps1", name=f"psy{t}_r{r}")
        for g in range(2):
            nc.tensor.matmul(
                psy[:],
                s["ots"][(t, g)][:],
                s["wo"][g][:],
                start=(g == 0),
                stop=(g == 1),
            )
        ysb = p_y.tile([128, 512], BF, tag="ysb")
        nc.vector.tensor_copy(ysb[:], psy[:])
        nc.sync.dma_start(y_d[t], ysb[:])

    def schedule_rep_prelude(r, base):
        """Schedule rep r's phase-1 + eb thunks into the stream.
        base = global unit index where scheduling may start."""
        pipe.push(base + 10, lambda: emit_in_dma(r))
        pipe.push(base + 20, lambda: emit_v_alloc(r))
        pipe.push(base + 20, lambda: emit_on_alloc(r))
        pipe.push(base + 21, lambda: emit_qk_alloc(r, 0))
        pipe.push(base + 21, lambda: emit_qk_alloc(r, 2))
        due = base + 22
        for tb in range(4):
            pipe.push(due, lambda ft=0, tb=tb: emit_qkv_group(r, ft, tb))
            due += 3
            pipe.push(due, lambda ft=2, tb=tb: emit_qkv_group(r, ft, tb))
            due += 3
        for tt in range(16):
            pipe.push(due, lambda tt=tt: emit_v_group(r, tt))
            due += 3
        # late halves (q/k heads 4-7): slots free after donor rep's hg0
        pipe.push(base + 66, lambda: emit_qk_alloc(r, 1))
        pipe.push(base + 67, lambda: emit_qk_alloc(r, 3))
        due = base + 68
        for tb in range(4):
            pipe.push(due, lambda ft=1, tb=tb: emit_qkv_group(r, ft, tb))
            due += 2
            pipe.push(due, lambda ft=3, tb=tb: emit_qkv_group(r, ft, tb))
            due += 2
        return due

    # ---- rep 0 prologue (direct emission, no pipeline) ---------------------
    emit_in_dma(0)
    emit_eb_dma(0, 0, 0)
    emit_eb_dma(0, 0, 1)
    emit_eb_dma(0, 1, 0)
    emit_v_alloc(0)
    emit_on_alloc(0)
    for ft in range(4):
        emit_qk_alloc(0, ft)
    for ft in range(4):
        for tb in range(4):
            emit_qkv_group(0, ft, tb)
    for tt in range(16):
        emit_v_group(0, tt)

    # ---- main rep loop -----------------------------------------------------
    for r in range(reps):
        R0 = r * UNITS
        if r == 0:
            # rep 0's e11 slot frees once hg0's readers drain
            pipe.push(R0 + 66, lambda r=r: emit_eb_dma(r, 1, 1))
        if r + 1 < reps:
            schedule_rep_prelude(r + 1, R0)
            # next rep's eb tiles: e00' after our e01 frees (hg0 done),
            # e01' after our e10 frees, e10' right after rep boundary,
            # e11' once next rep's hg0 drains.
            pipe.push(R0 + 68, lambda r=r: emit_eb_dma(r + 1, 0, 0))
            pipe.push(R0 + 124, lambda r=r: emit_eb_dma(r + 1, 0, 1))
            pipe.push(R0 + UNITS + 2, lambda r=r: emit_eb_dma(r + 1, 1, 0))
            pipe.push(R0 + UNITS + 66, lambda r=r: emit_eb_dma(r + 1, 1, 1))

        s = st[r]
        g = R0
        for hg in range(2):
            for b in range(BPC):
                for qh in range(2):
                    q0 = b * N + qh * 512
                    pv = p_pv.tile([128, 1024], F32, tag="pv", name=f"pv{hg}{b}{qh}_r{r}")
                    for kt in range(8):
                        half, ktl = kt // 4, kt % 4
                        ebtile = s[("eb", hg, half)]
                        for hp in range(2):
                            pd = p_pd.tile([128, 1024], F32, tag="pd", name="pd")
                            for jj in range(2):
                                j = hp * 2 + jj
                                pb = j * 32
                                nc.tensor.matmul(
                                    pd[:, jj * 512 : (jj + 1) * 512],
                                    s["qk"][2 + hg][pb : pb + 32, b * N + kt * 128 : b * N + kt * 128 + 128],
                                    s["qk"][hg][pb : pb + 32, q0 : q0 + 512],
                                    start=True,
                                    stop=True,
                                    tile_position=(pb, 0),
                                )
                            P = p_p.tile([128, 1024], BF, tag="P")
                            nc.scalar.activation(P[:], pd[:], Exp)
                            Ph = p_ph.tile([128, 1024], BF, tag="Ph")
                            ebsl = ebtile[:].rearrange("p (h q) -> p h q", h=16)[
                                :,
                                ktl * 4 + hp * 2 : ktl * 4 + hp * 2 + 2,
                                qh * 512 : qh * 512 + 512,
                            ]
                            psl = P[:].rearrange("p (h q) -> p h q", h=2)
                            phsl = Ph[:].rearrange("p (h q) -> p h q", h=2)
                            nc.vector.tensor_mul(phsl, psl, ebsl)
                            if (
                                dbg is not None
                                and r == reps - 1
                                and (hg, b, qh, kt, hp) == (0, 0, 0, 0, 0)
                            ):
                                pdsb = p_p.tile([128, 1024], BF, tag="P", name="pdsb")
                                nc.vector.tensor_copy(pdsb[:], pd[:])
                                nc.sync.dma_start(dbg["pd"], pdsb[:])
                                nc.sync.dma_start(dbg["P"], P[:])
                                nc.sync.dma_start(dbg["Ph"], Ph[:])
                            base = (b * 8 + kt) * 264
                            for jj in range(2):
                                j = hp * 2 + jj
                                h = hg * 4 + j
                                for c in range(4):
                                    # pv banks: bank hp holds heads hp*2..hp*2+1.
                                    # start=True only on the FIRST matmul to touch
                                    # a bank: start marks the whole 2KB zero region
                                    # pending-zero, so later footprints' first
                                    # writes overwrite (= init) and then accumulate.
                                    # Verified bit-exact on HW.
                                    nc.tensor.matmul(
                                        pv[:, j * 256 + c * 64 : j * 256 + c * 64 + 33],
                                        Ph[:, jj * 512 + c * 128 : jj * 512 + c * 128 + 128],
                                        s["v"][:, base + h * 33 : base + h * 33 + 33],
                                        start=(kt == 0 and jj == 0 and c == 0),
                                        stop=(kt == 7),
                                        skip_group_check=True,
                                    )
                            g += 1
                            pipe.tick(g)
                    # ---- merged normalize for this (hg, b, qh) -------------
                    if (
                        dbg is not None
                        and r == reps - 1
                        and (hg, b, qh) == (0, 0, 0)
                    ):
                        pvsb = p_p.tile([128, 1024], BF, tag="P", name="pvsb")
                        pv_in = dataclasses.replace(
                            pv[:, 0:33], ap=[list(pv[:].ap[0]), [64, 16], [1, 33]]
                        )
                        pv_out = dataclasses.replace(
                            pvsb[:, 0:33], ap=[list(pvsb[:].ap[0]), [33, 16], [1, 33]]
                        )
                        nc.vector.tensor_copy(pv_out, pv_in)
                        nc.sync.dma_start(dbg["pv"][:, 0:528], pvsb[:, 0:528])
                    rec = p_rec.tile([128, 16], F32, tag="rec")
                    pvap = pv[:]
                    den = dataclasses.replace(
                        pv[:, 32:33], ap=[list(pvap.ap[0]), [64, 4], [256, 4]]
                    )
                    nc.vector.reciprocal(
                        dataclasses.replace(
                            rec[:, 0:16], ap=[list(rec[:].ap[0]), [4, 4], [1, 4]]
                        ),
                        den,
                    )
                    t0c = (b * 8 + qh * 4) * 256 + hg * 128
                    in1 = dataclasses.replace(
                        pv[:, 0:32], ap=[list(pvap.ap[0]), [64, 4], [256, 4], [1, 32]]
                    )
                    in2 = dataclasses.replace(
                        rec[:, 0:16], ap=[list(rec[:].ap[0]), [4, 4], [1, 4], [0, 32]]
                    )
                    out = dataclasses.replace(
                        s["on"][:, t0c : t0c + 32],
                        ap=[list(s["on"][:].ap[0]), [256, 4], [32, 4], [1, 32]],
                    )
                    nc.vector.tensor_mul(out, in1, in2)
                # ---- phase 3 for batch b (after hg==1) ---------------------
                if hg == 1:
                    emit_p3_transposes(r, b)
                    for tloc in range(8):
                        t = b * 8 + tloc
                        if b == 0:
                            pipe.push(R0 + 98 + 2 * tloc, lambda r=r, t=t: emit_p3_chain(r, t))
                        else:
                            pipe.push(R0 + UNITS + 4 + 2 * tloc, lambda r=r, t=t: emit_p3_chain(r, t))
        pipe.tick(R0 + UNITS)
    pipe.flush()
    if dbg is not None:
        s = st[reps - 1]
        nc.sync.dma_start(dbg["on"], s["on"][:])
        nc.sync.dma_start(dbg["v"], s["v"][:])
        for ft in range(4):
            nc.sync.dma_start(dbg["qk"][ft], s["qk"][ft][:])


def build_program(reps=1, debug_outs=False):
    nc = bass.Bass("TRN2", target_bir_lowering=False, debug=False, num_devices=NCORES)
    xT_d = nc.dram_tensor("xT", [INP, T], BF, kind="ExternalInput").ap()
    wqk_d = nc.dram_tensor("wqk", [INP, 512], BF, kind="ExternalInput").ap()
    wv_d = nc.dram_tensor("wv", [INP, 256], BF, kind="ExternalInput").ap()
    eb_d = nc.dram_tensor("eb", [2, 2, 128, 16384], BF, kind="ExternalInput").ap()
    wo_d = nc.dram_tensor("wo", [2, 128, 512], BF, kind="ExternalInput").ap()
    y_d = nc.dram_tensor("y", [16, 128, OUP], BF, kind="ExternalOutput").ap()
    aps = (xT_d, wqk_d, wv_d, eb_d, wo_d, y_d)
    dbg = None
    if debug_outs:
        dbg = {
            "on": nc.dram_tensor("dbg_on", [128, 16 * 256], BF, kind="ExternalOutput").ap(),
            "v": nc.dram_tensor("dbg_v", [128, 16 * 264], BF, kind="ExternalOutput").ap(),
            "qk": [
                nc.dram_tensor(f"dbg_qk{i}", [128, T], BF, kind="ExternalOutput").ap()
                for i in range(4)
            ],
            "pd": nc.dram_tensor("dbg_pd", [128, 1024], BF, kind="ExternalOutput").ap(),
            "P": nc.dram_tensor("dbg_P", [128, 1024], BF, kind="ExternalOutput").ap(),
            "Ph": nc.dram_tensor("dbg_Ph", [128, 1024], BF, kind="ExternalOutput").ap(),
            "pv": nc.dram_tensor("dbg_pv", [128, 1024], BF, kind="ExternalOutput").ap(),
        }

    from contextlib import ExitStack

    with tile.TileContext(nc) as tc:
        with ExitStack() as es:
            _build(nc, tc, es, aps, reps, dbg=dbg)

    _split_waits(nc, cap=1)
    return nc


def _relative_index():
    ii, jj = np.meshgrid(np.arange(32), np.arange(32), indexing="ij")
    coords = np.stack([ii.reshape(-1), jj.reshape(-1)])
    rel = coords[:, :, None] - coords[:, None, :]
    return ((rel[0] + 31) * 63 + (rel[1] + 31)).reshape(-1)


def prepare_inputs(x, w_qkv, bias_table, w_out, b_out):
    """Host-side prep: returns per-core in_maps."""
    import ml_dtypes

    bf16 = ml_dtypes.bfloat16
    scale = DH ** -0.5

    wqk = np.ascontiguousarray(w_qkv[:, :512]).astype(np.float32).copy()
    wqk[:, :256] *= scale
    wqk = wqk.astype(bf16)
    wv = np.ascontiguousarray(w_qkv[:, 512:]).astype(bf16)

    idx = np.clip(_relative_index(), 0, TABLE - 1)
    bias = bias_table[idx].reshape(N, N, HEADS).astype(np.float32)  # [q, k, h]
    ebT = np.exp(bias).transpose(1, 0, 2)  # [k, q, h]
    eb = (
        ebT.reshape(2, 4, 128, N, 2, 4)  # [half, ktl, p, qt, hg, j]
        .transpose(4, 0, 2, 1, 5, 3)  # [hg, half, p, ktl, j, qt]
        .reshape(2, 2, 128, 16384)
        .astype(bf16)
    )

    wo = np.ascontiguousarray(w_out.astype(bf16)).reshape(2, 128, OUP)

    in_maps = []
    for c in range(NCORES):
        xc = x[c * BPC : (c + 1) * BPC].reshape(T, INP)
        xT = np.ascontiguousarray(xc.T).astype(bf16)
        in_maps.append({"xT": xT, "wqk": wqk, "wv": wv, "eb": eb, "wo": wo})
    return in_maps


_NC_CACHE = {}


def kernel(x, w_qkv, bias_table, w_out, b_out):
    in_maps = prepare_inputs(x, w_qkv, bias_table, w_out, b_out)
    if 1 not in _NC_CACHE:
        _NC_CACHE[1] = build_program(reps=1)
    nc = _NC_CACHE[1]
    res = run_bass_kernel_spmd(nc, in_maps, list(range(NCORES)), trace=False)
    y = np.concatenate(
        [
            res.results[c]["y"].astype(np.float32).reshape(T, OUP)
            for c in range(NCORES)
        ],
        axis=0,
    ).reshape(B, N, OUP)
    return (y + np.asarray(b_out, np.float32)[None, None, :]).astype(np.float32)
